# revision 2
# baseline (speedup 1.0000x reference)
"""DiagPooling (segment-reduce over square-image diagonals) on 8 NeuronCores.

Input  x: [8, 128, 512, 512] f32. Output: [8, 1, 513] f32 — per batch, the
mean over (channels, diagonal) of each diagonal offset in [-256, 256].

Sharding: batch b -> core b (data parallel, no communication).

Per-core pipeline (single-pass, no DRAM bounce):
1. Stream the 128 channels with partition p covering the flat range
   [2052*p, 2052*(p+1)) of each channel image (2052 = 4*513, one 8208-byte
   contiguous run per partition per channel). Because flat (i, j) = 513*i +
   (j - i) indexes the stride-513 diagonal view P[q, r] = y_flat[513*q + r],
   partition p of the accumulator then holds EXACTLY rows q = 4p..4p+3 of P
   — the channel sum lands pre-arranged for diagonal extraction, so the
   baseline's 2 MiB DRAM re-layout round-trip disappears. Runs that overhang
   a channel end (partition 127) read the next channel / a 512-element zero
   pad; those positions are provably masked out in step 2.
   The 32 four-channel 4 MiB loads alternate between the two HWDGE rings
   (sync + scalar) and accumulate on VectorE.
2. One masked multiply folds wanted(q, r) / (C * diag_len) into the
   accumulator; 3 DVE adds fold the 4 row-groups; two ones-vector matmuls
   (512 + 1 columns, PSUM-bank sized) give the 513 diagonal means.
"""

import numpy as np

import concourse.bass as bass
import concourse.bacc as bacc
import concourse.mybir as mybir
from concourse import tile
from concourse.bass_utils import run_bass_kernel_spmd

B, C, H = 8, 128, 512
R = H + 1               # 513 distinct wanted diagonals
T = 4                   # P-view rows per partition
F = T * R               # 2052: accumulator free width (= flat elems/partition)
CH_ELEMS = H * H        # 262144 elements per (b, c) image
N_IN = C * CH_ELEMS
PAD = F * 128 - CH_ELEMS  # 512: zero pad so partition 127's run stays in bounds
CH_PER = 4              # channels per stream DMA (4 MiB each)
NT = C // CH_PER        # 32 stream DMAs
F32 = mybir.dt.float32


def _mask_qr() -> np.ndarray:
    """[512, 513] f64: wanted(q, r) / (C * diag_len)."""
    q = np.arange(H, dtype=np.int64)[:, None]
    r = np.arange(R, dtype=np.int64)[None, :]
    prefix = (r <= H // 2) & (q + r <= H - 1)            # diagonal o = r
    suffix = (r > H // 2) & (q + r >= H) & (q <= H - 2)  # o = r - 513
    mask = prefix | suffix
    o = np.where(r <= H // 2, r, r - R)
    denom = float(C) * (H - np.abs(o)).astype(np.float64)
    return mask.astype(np.float64) / denom


def _build_weights() -> np.ndarray:
    """[128, F] f32: the mask in the accumulator layout
    (row q = 4*p + t -> partition p, free column t*513 + r)."""
    return _mask_qr().reshape(128, T, R).reshape(128, F).astype(np.float32)


def _build_program():
    nc = bacc.Bacc("TRN2", target_bir_lowering=False, debug=False, num_devices=B)
    xp = nc.dram_tensor("x", [N_IN + PAD], F32, kind="ExternalInput")
    wt = nc.dram_tensor("w", [128, F], F32, kind="ExternalInput")
    out_t = nc.dram_tensor("out", [1, R], F32, kind="ExternalOutput")

    NBUFS = 5

    with tile.TileContext(nc) as tc:
        with (
            tc.tile_pool(name="consts", bufs=1) as consts,
            tc.tile_pool(name="accp", bufs=1) as accp,
            tc.tile_pool(name="loadp", bufs=NBUFS) as loadp,
            tc.tile_pool(name="outp", bufs=1) as outp,
            tc.tile_pool(name="psum", bufs=2, space=bass.MemorySpace.PSUM) as psump,
        ):
            ones = consts.tile([128, 1], F32)
            nc.gpsimd.memset(ones[:], 1.0)

            # 1. channel stream in the diagonal-view layout, split over both
            # HWDGE rings; accumulate per channel slice on VectorE
            acc = accp.tile([128, F], F32)
            for k in range(NT):
                t = loadp.tile([128, CH_PER * F], F32)
                eng = nc.sync if k % 2 == 0 else nc.scalar
                eng.dma_start(
                    out=t[:],
                    in_=bass.AP(
                        xp,
                        k * CH_PER * CH_ELEMS,
                        [[F, 128], [CH_ELEMS, CH_PER], [1, F]],
                    ),
                )
                for c in range(CH_PER):
                    sl = t[:, c * F : (c + 1) * F]
                    if k == 0 and c == 0:
                        pass  # folded into the c == 1 add below
                    elif k == 0 and c == 1:
                        nc.vector.tensor_add(out=acc[:], in0=t[:, 0:F], in1=sl)
                    else:
                        nc.vector.tensor_add(out=acc[:], in0=acc[:], in1=sl)

            # mask weights ride the scalar ring BEHIND the stream: they land
            # during the final adds instead of delaying the first loads
            w_tile = consts.tile([128, F], F32)
            nc.scalar.dma_start(out=w_tile[:], in_=wt.ap())

            # 2. mask, fold the 4 row-groups, column-sum via ones matmuls
            nc.vector.tensor_mul(out=acc[:], in0=acc[:], in1=w_tile[:])
            u = outp.tile([128, R], F32)
            nc.vector.tensor_add(out=u[:], in0=acc[:, 0:R], in1=acc[:, R : 2 * R])
            nc.vector.tensor_add(out=u[:], in0=u[:], in1=acc[:, 2 * R : 3 * R])
            nc.vector.tensor_add(out=u[:], in0=u[:], in1=acc[:, 3 * R : 4 * R])
            ps_a = psump.tile([1, 512], F32)
            ps_b = psump.tile([1, 1], F32)
            nc.tensor.matmul(ps_a[:], ones[:], u[:, 0:512], start=True, stop=True)
            nc.tensor.matmul(ps_b[:], ones[:], u[:, 512:513], start=True, stop=True)
            res = outp.tile([1, R], F32)
            nc.vector.tensor_copy(out=res[:, 0:512], in_=ps_a[:])
            nc.vector.tensor_copy(out=res[:, 512:513], in_=ps_b[:])
            nc.sync.dma_start(out=out_t.ap(), in_=res[:])

    nc.compile()
    return nc


_CACHE = {}


def kernel(x, _trace=False, _trace_cores=None) -> np.ndarray:
    x = np.asarray(x, dtype=np.float32)
    assert x.shape == (B, C, H, H), x.shape

    if "nc" not in _CACHE:
        _CACHE["nc"] = _build_program()
        _CACHE["w"] = _build_weights()
    nc = _CACHE["nc"]
    w = _CACHE["w"]

    zpad = np.zeros(PAD, dtype=np.float32)
    in_maps = [
        {
            "x": np.concatenate([np.ascontiguousarray(x[b]).reshape(-1), zpad]),
            "w": w,
        }
        for b in range(B)
    ]
    result = run_bass_kernel_spmd(
        nc,
        in_maps,
        core_ids=list(range(B)),
        trace=_trace,
        trace_cores=_trace_cores,
    )
    _CACHE["last_result"] = result

    out = np.empty((B, 1, R), dtype=np.float32)
    for b in range(B):
        r = result.results[b]["out"].reshape(R)
        # column r -> offset o = r (r <= 256) / r - 513 (r >= 257);
        # output index n = o + 256
        out[b, 0, :] = np.concatenate([r[R - 256 :], r[: R - 256]])
    return out


# revision 6
# speedup vs baseline: 1.0394x; 1.0394x over previous
"""DiagPooling (segment-reduce over square-image diagonals) on 8 NeuronCores.

Input  x: [8, 128, 512, 512] f32. Output: [8, 1, 513] f32 — per batch, the
mean over (channels, diagonal) of each diagonal offset in [-256, 256].

Sharding: batch b -> core b (data parallel, no communication).

Per-core pipeline (single-pass, no DRAM bounce):
1. Stream the 128 channels with partition p covering the flat range
   [2052*p, 2052*(p+1)) of each channel image (2052 = 4*513, one 8208-byte
   contiguous run per partition per channel). Because flat (i, j) = 513*i +
   (j - i) indexes the stride-513 diagonal view P[q, r] = y_flat[513*q + r],
   partition p of the accumulator then holds EXACTLY rows q = 4p..4p+3 of P
   — the channel sum lands pre-arranged for diagonal extraction, so the
   baseline's 2 MiB DRAM re-layout round-trip disappears. Runs that overhang
   a channel end (partition 127) read the next channel / a 512-element zero
   pad; those positions are provably masked out in step 2.
   The 128 per-channel 1 MiB loads alternate between the two HWDGE rings
   (sync + scalar) and accumulate on VectorE. Load tiles are padded to a
   64-byte-multiple pitch: a 8208-byte operand offset costs DVE ~20%
   (measured 2754 vs 2292 ns per add), so every add operand must stay
   64B-aligned.
2. One masked multiply folds wanted(q, r) / (C * diag_len) into the
   accumulator; 3 DVE adds fold the 4 row-groups; two ones-vector matmuls
   (512 + 1 columns, PSUM-bank sized) give the 513 diagonal means.
"""

import numpy as np

import concourse.bass as bass
import concourse.bacc as bacc
import concourse.mybir as mybir
from concourse import tile
from concourse.bass_utils import run_bass_kernel_spmd

B, C, H = 8, 128, 512
R = H + 1               # 513 distinct wanted diagonals
T = 4                   # P-view rows per partition
F = T * R               # 2052: accumulator free width (= flat elems/partition)
CH_ELEMS = H * H        # 262144 elements per (b, c) image
N_IN = C * CH_ELEMS
PAD = F * 128 - CH_ELEMS  # 512: zero pad so partition 127's run stays in bounds
FP = 2064               # load-tile pitch: 8256 B = 129*64 keeps slots 64B-aligned
F32 = mybir.dt.float32


def _mask_qr() -> np.ndarray:
    """[512, 513] f64: wanted(q, r) / (C * diag_len)."""
    q = np.arange(H, dtype=np.int64)[:, None]
    r = np.arange(R, dtype=np.int64)[None, :]
    prefix = (r <= H // 2) & (q + r <= H - 1)            # diagonal o = r
    suffix = (r > H // 2) & (q + r >= H) & (q <= H - 2)  # o = r - 513
    mask = prefix | suffix
    o = np.where(r <= H // 2, r, r - R)
    denom = float(C) * (H - np.abs(o)).astype(np.float64)
    return mask.astype(np.float64) / denom


def _build_weights() -> np.ndarray:
    """[128, F] f32: the mask in the accumulator layout
    (row q = 4*p + t -> partition p, free column t*513 + r)."""
    return _mask_qr().reshape(128, T, R).reshape(128, F).astype(np.float32)


def _build_program():
    nc = bacc.Bacc("TRN2", target_bir_lowering=False, debug=False, num_devices=B)
    xp = nc.dram_tensor("x", [N_IN + PAD], F32, kind="ExternalInput")
    wt = nc.dram_tensor("w", [128, F], F32, kind="ExternalInput")
    out_t = nc.dram_tensor("out", [1, R], F32, kind="ExternalOutput")

    NBUFS = 12

    with tile.TileContext(nc) as tc:
        with (
            tc.tile_pool(name="consts", bufs=1) as consts,
            tc.tile_pool(name="accp", bufs=1) as accp,
            tc.tile_pool(name="loadp", bufs=NBUFS) as loadp,
            tc.tile_pool(name="outp", bufs=1) as outp,
            tc.tile_pool(name="psum", bufs=2, space=bass.MemorySpace.PSUM) as psump,
        ):
            ones = consts.tile([128, 1], F32)
            nc.gpsimd.memset(ones[:], 1.0)

            # 1. channel stream in the diagonal-view layout, split over both
            # HWDGE rings; accumulate per channel on VectorE
            acc = accp.tile([128, F], F32)
            prev = None
            for c in range(C):
                t = loadp.tile([128, FP], F32)
                eng = nc.sync if c % 2 == 0 else nc.scalar
                eng.dma_start(
                    out=t[:, 0:F],
                    in_=bass.AP(xp, c * CH_ELEMS, [[F, 128], [1, F]]),
                )
                if c == 0:
                    prev = t
                elif c == 1:
                    nc.vector.tensor_add(
                        out=acc[:], in0=prev[:, 0:F], in1=t[:, 0:F]
                    )
                else:
                    nc.vector.tensor_add(out=acc[:], in0=acc[:], in1=t[:, 0:F])

            # mask weights ride the scalar ring BEHIND the stream: they land
            # during the final adds instead of delaying the first loads
            w_tile = consts.tile([128, F], F32)
            nc.scalar.dma_start(out=w_tile[:], in_=wt.ap())

            # 2. mask, fold the 4 row-groups, column-sum via ones matmuls
            nc.vector.tensor_mul(out=acc[:], in0=acc[:], in1=w_tile[:])
            u = outp.tile([128, R], F32)
            nc.vector.tensor_add(out=u[:], in0=acc[:, 0:R], in1=acc[:, R : 2 * R])
            nc.vector.tensor_add(out=u[:], in0=u[:], in1=acc[:, 2 * R : 3 * R])
            nc.vector.tensor_add(out=u[:], in0=u[:], in1=acc[:, 3 * R : 4 * R])
            ps_a = psump.tile([1, 512], F32)
            ps_b = psump.tile([1, 1], F32)
            nc.tensor.matmul(ps_a[:], ones[:], u[:, 0:512], start=True, stop=True)
            nc.tensor.matmul(ps_b[:], ones[:], u[:, 512:513], start=True, stop=True)
            res = outp.tile([1, R], F32)
            nc.vector.tensor_copy(out=res[:, 0:512], in_=ps_a[:])
            nc.vector.tensor_copy(out=res[:, 512:513], in_=ps_b[:])
            nc.sync.dma_start(out=out_t.ap(), in_=res[:])

    nc.compile()
    return nc


_CACHE = {}


def kernel(x, _trace=False, _trace_cores=None) -> np.ndarray:
    x = np.asarray(x, dtype=np.float32)
    assert x.shape == (B, C, H, H), x.shape

    if "nc" not in _CACHE:
        _CACHE["nc"] = _build_program()
        _CACHE["w"] = _build_weights()
    nc = _CACHE["nc"]
    w = _CACHE["w"]

    zpad = np.zeros(PAD, dtype=np.float32)
    in_maps = [
        {
            "x": np.concatenate([np.ascontiguousarray(x[b]).reshape(-1), zpad]),
            "w": w,
        }
        for b in range(B)
    ]
    result = run_bass_kernel_spmd(
        nc,
        in_maps,
        core_ids=list(range(B)),
        trace=_trace,
        trace_cores=_trace_cores,
    )
    _CACHE["last_result"] = result

    out = np.empty((B, 1, R), dtype=np.float32)
    for b in range(B):
        r = result.results[b]["out"].reshape(R)
        # column r -> offset o = r (r <= 256) / r - 513 (r >= 257);
        # output index n = o + 256
        out[b, 0, :] = np.concatenate([r[R - 256 :], r[: R - 256]])
    return out
